# revision 6
# baseline (speedup 1.0000x reference)
"""Trainium2 Bass kernel: batched causal attention (B=4, S=4096, E=256, f32).

Sharding: 2 cores per batch element (4 pairs). Within a pair, the key/value
rows are split even/odd at 128-row tile granularity; both cores process all
4096 query rows of their batch against their 2048 K/V rows.  This makes the
SPMD instruction stream identical across cores (only data differs) and
perfectly load-balances the causal structure.  Partial (P@V, rowsum) results
are merged across each pair with a ReduceScatter, after which each core
normalizes and writes half the batch rows.

Compute layout (per core):
  X^T, Z^T via PE transposes -> Q^T = WqT @ X^T (scaled by 1/sqrt(E), +bq),
  K^T = WkT @ Z^T (bk dropped: softmax shift-invariant), V = Z^T(stationary)
  @ WvT (bv folded in at the end: attn rows sum to 1).
  Scores computed transposed per tile: S^T[k,q] = K^T(stationary) . Q^T, so
  exp(PSUM)->SBUF directly yields P^T for the PV matmul (no P transpose).
  Rowsums via an M=1 ones-matmul.  Matmuls run in float32r (~1e-4 rel err,
  4x faster than plain f32 on the PE).
"""

import numpy as np

B = 4
S = 4096
E = 256
SK = S // 2          # K/V rows per core
KT = SK // 128       # 16 local k-tiles
NCHUNK = S // 512    # 8 q-chunks of 512
F = 512              # q free dim per chunk
NPOST = NCHUNK // 2  # post-phase chunks per core

_COMPILED = {}


def _build():
    import concourse.bass as bass
    import concourse.tile as tile
    from concourse import mybir, bacc

    f32 = mybir.dt.float32
    f32r = mybir.dt.float32r
    Exp = mybir.ActivationFunctionType.Exp
    Copy = mybir.ActivationFunctionType.Copy
    Ident = mybir.ActivationFunctionType.Identity

    nc = bacc.Bacc("TRN2", target_bir_lowering=False, debug=False,
                   enable_asserts=True, num_devices=8)

    x_ext = nc.dram_tensor("x", [S, E], f32, kind="ExternalInput")
    z_ext = nc.dram_tensor("z", [SK, E], f32, kind="ExternalInput")
    wq_ext = nc.dram_tensor("wq", [E, E], f32, kind="ExternalInput")
    wk_ext = nc.dram_tensor("wk", [E, E], f32, kind="ExternalInput")
    wv_ext = nc.dram_tensor("wv", [E, E], f32, kind="ExternalInput")
    bqs_ext = nc.dram_tensor("bqs", [E], f32, kind="ExternalInput")  # bq/sqrt(E)
    bv_ext = nc.dram_tensor("bv", [E], f32, kind="ExternalInput")
    masks_ext = nc.dram_tensor("masks", [2, 128, F], f32, kind="ExternalInput")
    ones_ext = nc.dram_tensor("ones", [128, 128], f32, kind="ExternalInput")
    out_ext = nc.dram_tensor("out", [S // 2, E], f32, kind="ExternalOutput")

    from concourse.masks import make_identity

    with tile.TileContext(nc) as tc:
        with tc.tile_pool(name="singles", bufs=1) as singles, \
             tc.tile_pool(name="dram", bufs=1, space="DRAM") as dram:
            # ---- constants -------------------------------------------------
            ident = singles.tile([128, 128], f32)
            make_identity(nc, ident[:])
            ones_r = singles.tile([128, 128], f32r)
            nc.gpsimd.dma_start(out=ones_r[:], in_=ones_ext[:])
            maskt = singles.tile([128, 2, F], f32r)
            nc.gpsimd.dma_start(out=maskt[:], in_=masks_ext.ap().rearrange("m p f -> p m f"))
            bqs = singles.tile([128, 2], f32)  # [:, ft] per-partition bias
            for ft in range(2):
                nc.sync.dma_start(out=bqs[:, ft:ft + 1],
                                  in_=bqs_ext[128 * ft:128 * (ft + 1)].rearrange("(p one) -> p one", one=1))
            bv_bc = singles.tile([128, E], f32)
            nc.sync.dma_start(
                out=bv_bc[:],
                in_=bass.AP(tensor=bv_ext, offset=0, ap=[[0, 128], [1, E]]))

            # ---- weights: W^T[e', f] in SBUF (f32r), via PE transposes -----
            wT = {}
            with tc.tile_pool(name="wload", bufs=2) as wload, \
                 tc.tile_pool(name="ps_w", bufs=2, space="PSUM") as ps_w:
                for wname, wext in (("q", wq_ext), ("k", wk_ext), ("v", wv_ext)):
                    for et in range(2):
                        wT[wname, et] = singles.tile([128, E], f32r, name=f"wT_{wname}{et}")
                    for ft in range(2):
                        wnat = wload.tile([128, E], f32)
                        nc.sync.dma_start(out=wnat[:],
                                          in_=wext[128 * ft:128 * (ft + 1), :])
                        pst = ps_w.tile([128, E], f32)
                        for et in range(2):
                            nc.tensor.transpose(pst[:, 128 * et:128 * (et + 1)],
                                                wnat[:, 128 * et:128 * (et + 1)],
                                                ident[:])
                        for et in range(2):
                            nc.vector.tensor_copy(
                                out=wT[wname, et][:, 128 * ft:128 * (ft + 1)],
                                in_=pst[:, 128 * et:128 * (et + 1)])

            # ---- big persistent SBUF tensors -------------------------------
            qT = [singles.tile([128, S], f32r, name=f"qT{i}", tag=f"qT{i}") for i in range(2)]
            kT = [singles.tile([128, SK], f32r, name=f"kT{i}", tag=f"kT{i}") for i in range(2)]
            v_sb = singles.tile([128, KT, E], f32r, tag="v_sb")

            # ---- projections ----------------------------------------------
            with tc.tile_pool(name="nat", bufs=3) as nat, \
                 tc.tile_pool(name="trsb", bufs=4) as trsb, \
                 tc.tile_pool(name="ps_tr", bufs=3, space="PSUM") as ps_tr, \
                 tc.tile_pool(name="ps_mm", bufs=3, space="PSUM") as ps_mm:
                # X^T chunks -> Q^T
                for sc in range(NCHUNK):
                    x_nat = nat.tile([128, 4, E], f32)
                    nc.sync.dma_start(
                        out=x_nat[:],
                        in_=x_ext[512 * sc:512 * (sc + 1), :].rearrange(
                            "(t p) e -> p t e", p=128))
                    xT = []
                    for et in range(2):
                        pst = ps_tr.tile([128, F], f32, tag="ps_tr")
                        for t in range(4):
                            nc.tensor.transpose(
                                pst[:, 128 * t:128 * (t + 1)],
                                x_nat[:, t, 128 * et:128 * (et + 1)], ident[:])
                        xt = trsb.tile([128, F], f32r, tag="xT")
                        nc.vector.tensor_copy(out=xt[:], in_=pst[:])
                        xT.append(xt)
                    for ft in range(2):
                        psq = ps_mm.tile([128, F], f32, tag="ps_mm")
                        for et in range(2):
                            nc.tensor.matmul(psq[:], wT["q", et][:, 128 * ft:128 * (ft + 1)],
                                             xT[et][:], start=(et == 0), stop=(et == 1))
                        nc.scalar.activation(out=qT[ft][:, 512 * sc:512 * (sc + 1)],
                                             in_=psq[:], func=Ident,
                                             bias=bqs[:, ft:ft + 1],
                                             scale=1.0 / np.sqrt(E))
                # Z^T chunks -> K^T, V
                for sc in range(4):
                    z_nat = nat.tile([128, 4, E], f32, tag="x_nat")
                    nc.sync.dma_start(
                        out=z_nat[:],
                        in_=z_ext[512 * sc:512 * (sc + 1), :].rearrange(
                            "(t p) e -> p t e", p=128))
                    zT = []
                    for et in range(2):
                        pst = ps_tr.tile([128, F], f32, tag="ps_tr")
                        for t in range(4):
                            nc.tensor.transpose(
                                pst[:, 128 * t:128 * (t + 1)],
                                z_nat[:, t, 128 * et:128 * (et + 1)], ident[:])
                        zt = trsb.tile([128, F], f32r, tag="xT")
                        nc.vector.tensor_copy(out=zt[:], in_=pst[:])
                        zT.append(zt)
                    for ft in range(2):
                        psk = ps_mm.tile([128, F], f32, tag="ps_mm")
                        for et in range(2):
                            nc.tensor.matmul(psk[:], wT["k", et][:, 128 * ft:128 * (ft + 1)],
                                             zT[et][:], start=(et == 0), stop=(et == 1))
                        nc.scalar.activation(out=kT[ft][:, 512 * sc:512 * (sc + 1)],
                                             in_=psk[:], func=Copy)
                    for t in range(4):
                        psv = ps_mm.tile([128, E], f32, tag="ps_mm")
                        for et in range(2):
                            nc.tensor.matmul(psv[:], zT[et][:, 128 * t:128 * (t + 1)],
                                             wT["v", et][:], start=(et == 0), stop=(et == 1))
                        nc.scalar.activation(out=v_sb[:, 4 * sc + t, :], in_=psv[:],
                                             func=Copy)

            # ---- attention -------------------------------------------------
            partials_in = dram.tile([NCHUNK, 257, F], f32)
            partials_out = dram.tile([NPOST, 257, F], f32)

            with tc.tile_pool(name="pT", bufs=3) as pTp, \
                 tc.tile_pool(name="partsb", bufs=2) as partsb, \
                 tc.tile_pool(name="ps_s", bufs=2, space="PSUM") as ps_s, \
                 tc.tile_pool(name="ps_o", bufs=2, space="PSUM") as ps_o, \
                 tc.tile_pool(name="ps_rs", bufs=2, space="PSUM") as ps_rs:
                for j in range(NCHUNK):
                    nkt = 2 * (j + 1)
                    pso = ps_o.tile([128, 2 * F], f32, tag="ps_o")
                    psr = ps_rs.tile([128, F], f32, tag="ps_rs")
                    for ll in range(nkt):
                        pss = ps_s.tile([128, F], f32, tag="ps_s")
                        for et in range(2):
                            nc.tensor.matmul(pss[:], kT[et][:, 128 * ll:128 * (ll + 1)],
                                             qT[et][:, 512 * j:512 * (j + 1)],
                                             start=(et == 0), stop=(et == 1))
                        pT = pTp.tile([128, F], f32r, tag="pT")
                        nc.scalar.activation(out=pT[:], in_=pss[:], func=Exp)
                        if ll >= nkt - 2:
                            nc.vector.tensor_mul(pT[:], pT[:],
                                                 maskt[:, ll - (nkt - 2), :])
                        for ft in range(2):
                            nc.tensor.matmul(pso[:, F * ft:F * (ft + 1)],
                                             v_sb[:, ll, 128 * ft:128 * (ft + 1)],
                                             pT[:], start=(ll == 0), stop=(ll == nkt - 1),
                                             skip_group_check=True)
                        nc.tensor.matmul(psr[:], ones_r[:], pT[:],
                                         start=(ll == 0), stop=(ll == nkt - 1),
                                         skip_group_check=True)
                    po_sb = partsb.tile([128, 2 * F], f32, tag="po_sb")
                    nc.scalar.activation(out=po_sb[:], in_=pso[:], func=Copy)
                    pr_sb = partsb.tile([1, F], f32, tag="pr_sb")
                    nc.vector.tensor_copy(out=pr_sb[:], in_=psr[0:1, :])
                    for ft in range(2):
                        nc.sync.dma_start(
                            out=partials_in[j, 128 * ft:128 * (ft + 1), :],
                            in_=po_sb[:, F * ft:F * (ft + 1)])
                    nc.sync.dma_start(out=partials_in[j, 256, :], in_=pr_sb[0:1, :])

                nc.gpsimd.collective_compute(
                    "ReduceScatter", mybir.AluOpType.add,
                    replica_groups=[[0, 1], [2, 3], [4, 5], [6, 7]],
                    ins=[partials_in.opt()],
                    outs=[partials_out.opt()])

            # ---- post: normalize, transpose back, +bv, store ---------------
            with tc.tile_pool(name="post", bufs=2) as post, \
                 tc.tile_pool(name="ps_po", bufs=4, space="PSUM") as ps_po:
                for c in range(NPOST):
                    oT_sb = post.tile([128, 2 * F], f32, tag="oT_sb")
                    for ft in range(2):
                        nc.sync.dma_start(out=oT_sb[:, F * ft:F * (ft + 1)],
                                          in_=partials_out[c, 128 * ft:128 * (ft + 1), :])
                    rs_ld = post.tile([128, 4], f32, tag="rs_ld")
                    nc.sync.dma_start(out=rs_ld[:],
                                      in_=partials_out[c, 256, :].rearrange("(t p) -> p t", p=128))
                    rs_t = post.tile([128, 4], f32, tag="rs_t")
                    nc.vector.reciprocal(out=rs_t[:], in_=rs_ld[:])
                    onat = post.tile([128, 4, E], f32, tag="onat")
                    for t in range(4):
                        pst = ps_po.tile([128, E], f32, tag="ps_po")
                        for ft in range(2):
                            nc.tensor.transpose(
                                pst[:, 128 * ft:128 * (ft + 1)],
                                oT_sb[:, F * ft + 128 * t:F * ft + 128 * (t + 1)],
                                ident[:])
                        nc.scalar.activation(out=onat[:, t, :], in_=pst[:],
                                             func=Copy, scale=rs_t[:, t:t + 1])
                        nc.vector.tensor_add(onat[:, t, :], onat[:, t, :], bv_bc[:])
                    nc.sync.dma_start(
                        out=out_ext[512 * c:512 * (c + 1), :].rearrange(
                            "(t p) e -> p t e", p=128),
                        in_=onat[:])

    nc.compile()
    return nc


def _get_nc():
    if "nc" not in _COMPILED:
        _COMPILED["nc"] = _build()
    return _COMPILED["nc"]


def kernel(X, Z, mask, Wq, bq, Wk, bk, Wv, bv):
    X = np.asarray(X, dtype=np.float32)
    Z = np.asarray(Z, dtype=np.float32)
    mask_np = np.asarray(mask)

    causal = bool(np.array_equal(
        mask_np != 0, np.tril(np.ones((S, S), dtype=bool))))
    if not causal:
        return _numpy_ref(X, Z, mask_np, Wq, bq, Wk, bk, Wv, bv)

    from concourse.bass_utils import run_bass_kernel_spmd

    nc = _get_nc()

    Wq = np.ascontiguousarray(Wq, dtype=np.float32)
    Wk = np.ascontiguousarray(Wk, dtype=np.float32)
    Wv = np.ascontiguousarray(Wv, dtype=np.float32)
    bqs = (np.asarray(bq, dtype=np.float32) / np.float32(np.sqrt(E))).copy()
    bv = np.ascontiguousarray(bv, dtype=np.float32)
    ones = np.ones((128, 128), dtype=np.float32)

    # masks per parity: last-2 local k-tiles of each chunk; keep iff y >= x+d
    y = np.arange(F)[None, :]
    x = np.arange(128)[:, None]
    masks_par = []
    for p in range(2):
        m = np.stack([(y >= x + 128 * p).astype(np.float32),
                      (y >= x + 256 + 128 * p).astype(np.float32)])
        masks_par.append(np.ascontiguousarray(m))

    in_maps = []
    for c in range(8):
        b, p = c // 2, c % 2
        zb = Z[b].reshape(S // 128, 128, E)
        z_shard = np.ascontiguousarray(zb[p::2].reshape(SK, E))
        in_maps.append({
            "x": np.ascontiguousarray(X[b]),
            "z": z_shard,
            "wq": Wq, "wk": Wk, "wv": Wv,
            "bqs": bqs, "bv": bv,
            "masks": masks_par[p],
            "ones": ones,
        })

    res = run_bass_kernel_spmd(nc, in_maps, core_ids=list(range(8)))

    out = np.empty((B, S, E), dtype=np.float32)
    for b in range(B):
        out[b, :S // 2] = res.results[2 * b]["out"]
        out[b, S // 2:] = res.results[2 * b + 1]["out"]
    return out


def _numpy_ref(X, Z, mask, Wq, bq, Wk, bk, Wv, bv):
    q = np.einsum("bse,fe->bsf", X, Wq) + bq
    k = np.einsum("bse,fe->bsf", Z, Wk) + bk
    v = np.einsum("bse,fe->bsf", Z, Wv) + bv
    s = np.einsum("bqe,bke->bqk", q, k) / np.sqrt(np.float32(X.shape[-1]))
    s = np.where(mask == 0, -np.inf, s)
    s = s - s.max(axis=-1, keepdims=True)
    p = np.exp(s)
    p /= p.sum(axis=-1, keepdims=True)
    return np.einsum("bqk,bke->bqe", p, v).astype(np.float32)


# revision 8
# speedup vs baseline: 1.2645x; 1.2645x over previous
"""Trainium2 Bass kernel: batched causal attention (B=4, S=4096, E=256, f32).

Sharding: 2 cores per batch element (4 pairs). Within a pair, the key/value
rows are split even/odd at 128-row tile granularity; both cores process all
4096 query rows of their batch against their 2048 K/V rows.  This makes the
SPMD instruction stream identical across cores (only data differs) and
perfectly load-balances the causal structure.  Partial (P@V, rowsum) results
are merged across each pair with a ReduceScatter, after which each core
normalizes and writes half the batch rows.

Compute layout (per core):
  X^T, Z^T via PE transposes -> Q^T = WqT @ X^T (scaled by 1/sqrt(E), +bq),
  K^T = WkT @ Z^T (bk dropped: softmax shift-invariant), V = Z^T(stationary)
  @ WvT (bv folded in at the end: attn rows sum to 1).
  Scores computed transposed per tile: S^T[k,q] = K^T(stationary) . Q^T, so
  exp(PSUM)->SBUF directly yields P^T for the PV matmul (no P transpose).
  Rowsums via an M=1 ones-matmul.  Matmuls run in float32r (~1e-4 rel err,
  4x faster than plain f32 on the PE).
"""

import numpy as np

B = 4
S = 4096
E = 256
SK = S // 2          # K/V rows per core
KT = SK // 128       # 16 local k-tiles
NCHUNK = S // 512    # 8 q-chunks of 512
F = 512              # q free dim per chunk
NPOST = NCHUNK // 2  # post-phase chunks per core

_COMPILED = {}


def _build():
    import concourse.bass as bass
    import concourse.tile as tile
    from concourse import mybir, bacc

    f32 = mybir.dt.float32
    f32r = mybir.dt.float32r  # noqa
    bf16 = mybir.dt.bfloat16
    Exp = mybir.ActivationFunctionType.Exp
    Copy = mybir.ActivationFunctionType.Copy
    Ident = mybir.ActivationFunctionType.Identity

    nc = bacc.Bacc("TRN2", target_bir_lowering=False, debug=False,
                   enable_asserts=True, num_devices=8)

    x_ext = nc.dram_tensor("x", [S, E], f32, kind="ExternalInput")
    z_ext = nc.dram_tensor("z", [SK, E], f32, kind="ExternalInput")
    wq_ext = nc.dram_tensor("wq", [E, E], f32, kind="ExternalInput")
    wk_ext = nc.dram_tensor("wk", [E, E], f32, kind="ExternalInput")
    wv_ext = nc.dram_tensor("wv", [E, E], f32, kind="ExternalInput")
    bqs_ext = nc.dram_tensor("bqs", [E], f32, kind="ExternalInput")  # bq/sqrt(E)
    bv_ext = nc.dram_tensor("bv", [E], f32, kind="ExternalInput")
    masks_ext = nc.dram_tensor("masks", [2, 128, F], f32, kind="ExternalInput")
    ones_ext = nc.dram_tensor("ones", [128, 128], f32, kind="ExternalInput")
    out_ext = nc.dram_tensor("out", [S // 2, E], f32, kind="ExternalOutput")

    from concourse.masks import make_identity

    with tile.TileContext(nc) as tc:
        with tc.tile_pool(name="singles", bufs=1) as singles, \
             tc.tile_pool(name="dram", bufs=1, space="DRAM") as dram:
            # ---- constants -------------------------------------------------
            ident = singles.tile([128, 128], f32)
            make_identity(nc, ident[:])
            ident_bf = singles.tile([128, 128], bf16)
            make_identity(nc, ident_bf[:])
            ones_r = singles.tile([128, 128], bf16)
            nc.gpsimd.dma_start(out=ones_r[:], in_=ones_ext[:])
            maskt = singles.tile([128, 2, F], bf16)
            nc.gpsimd.dma_start(out=maskt[:], in_=masks_ext.ap().rearrange("m p f -> p m f"))
            bqs = singles.tile([128, 2], f32)  # [:, ft] per-partition bias
            for ft in range(2):
                nc.sync.dma_start(out=bqs[:, ft:ft + 1],
                                  in_=bqs_ext[128 * ft:128 * (ft + 1)].rearrange("(p one) -> p one", one=1))
            bv_bc = singles.tile([128, E], f32)
            nc.sync.dma_start(
                out=bv_bc[:],
                in_=bass.AP(tensor=bv_ext, offset=0, ap=[[0, 128], [1, E]]))

            # ---- weights: W^T[e', f] in SBUF (f32r), via PE transposes -----
            wT = {}
            with tc.tile_pool(name="wload", bufs=2) as wload, \
                 tc.tile_pool(name="ps_w", bufs=2, space="PSUM") as ps_w:
                for wname, wext in (("q", wq_ext), ("k", wk_ext), ("v", wv_ext)):
                    for et in range(2):
                        wT[wname, et] = singles.tile([128, E], bf16, name=f"wT_{wname}{et}")
                    for ft in range(2):
                        wnat = wload.tile([128, E], bf16)
                        nc.gpsimd.dma_start(out=wnat[:],
                                            in_=wext[128 * ft:128 * (ft + 1), :])
                        pst = ps_w.tile([128, E], bf16)
                        for et in range(2):
                            nc.tensor.transpose(pst[:, 128 * et:128 * (et + 1)],
                                                wnat[:, 128 * et:128 * (et + 1)],
                                                ident_bf[:])
                        for et in range(2):
                            nc.vector.tensor_copy(
                                out=wT[wname, et][:, 128 * ft:128 * (ft + 1)],
                                in_=pst[:, 128 * et:128 * (et + 1)])

            # ---- big persistent SBUF tensors -------------------------------
            qT = [singles.tile([128, S], bf16, name=f"qT{i}", tag=f"qT{i}") for i in range(2)]
            kT = [singles.tile([128, SK], bf16, name=f"kT{i}", tag=f"kT{i}") for i in range(2)]
            v_sb = singles.tile([128, KT, E], bf16, tag="v_sb")

            # ---- projections ----------------------------------------------
            with tc.tile_pool(name="nat", bufs=3) as nat, \
                 tc.tile_pool(name="trsb", bufs=4) as trsb, \
                 tc.tile_pool(name="ps_tr", bufs=3, space="PSUM") as ps_tr, \
                 tc.tile_pool(name="ps_mm", bufs=3, space="PSUM") as ps_mm:
                # X^T chunks -> Q^T
                for sc in range(NCHUNK):
                    x_nat = nat.tile([128, 4, E], bf16)
                    nc.gpsimd.dma_start(
                        out=x_nat[:],
                        in_=x_ext[512 * sc:512 * (sc + 1), :].rearrange(
                            "(t p) e -> p t e", p=128))
                    xT = []
                    for et in range(2):
                        pst = ps_tr.tile([128, F], bf16, tag="ps_tr")
                        for t in range(4):
                            nc.tensor.transpose(
                                pst[:, 128 * t:128 * (t + 1)],
                                x_nat[:, t, 128 * et:128 * (et + 1)], ident_bf[:])
                        xt = trsb.tile([128, F], bf16, tag="xT")
                        nc.vector.tensor_copy(out=xt[:], in_=pst[:])
                        xT.append(xt)
                    for ft in range(2):
                        psq = ps_mm.tile([128, F], f32, tag="ps_mm")
                        for et in range(2):
                            nc.tensor.matmul(psq[:], wT["q", et][:, 128 * ft:128 * (ft + 1)],
                                             xT[et][:], start=(et == 0), stop=(et == 1))
                        nc.scalar.activation(out=qT[ft][:, 512 * sc:512 * (sc + 1)],
                                             in_=psq[:], func=Ident,
                                             bias=bqs[:, ft:ft + 1],
                                             scale=1.0 / np.sqrt(E))
                # Z^T chunks -> K^T, V
                for sc in range(4):
                    z_nat = nat.tile([128, 4, E], bf16, tag="x_nat")
                    nc.gpsimd.dma_start(
                        out=z_nat[:],
                        in_=z_ext[512 * sc:512 * (sc + 1), :].rearrange(
                            "(t p) e -> p t e", p=128))
                    zT = []
                    for et in range(2):
                        pst = ps_tr.tile([128, F], bf16, tag="ps_tr")
                        for t in range(4):
                            nc.tensor.transpose(
                                pst[:, 128 * t:128 * (t + 1)],
                                z_nat[:, t, 128 * et:128 * (et + 1)], ident_bf[:])
                        zt = trsb.tile([128, F], bf16, tag="xT")
                        nc.vector.tensor_copy(out=zt[:], in_=pst[:])
                        zT.append(zt)
                    for ft in range(2):
                        psk = ps_mm.tile([128, F], f32, tag="ps_mm")
                        for et in range(2):
                            nc.tensor.matmul(psk[:], wT["k", et][:, 128 * ft:128 * (ft + 1)],
                                             zT[et][:], start=(et == 0), stop=(et == 1))
                        nc.scalar.activation(out=kT[ft][:, 512 * sc:512 * (sc + 1)],
                                             in_=psk[:], func=Copy)
                    for t in range(4):
                        psv = ps_mm.tile([128, E], f32, tag="ps_mm")
                        for et in range(2):
                            nc.tensor.matmul(psv[:], zT[et][:, 128 * t:128 * (t + 1)],
                                             wT["v", et][:], start=(et == 0), stop=(et == 1))
                        nc.scalar.activation(out=v_sb[:, 4 * sc + t, :], in_=psv[:],
                                             func=Copy)

            # ---- attention -------------------------------------------------
            partials_in = dram.tile([NPOST, 2, 257, F], f32)
            partials_out = dram.tile([NPOST, 257, F], f32)

            with tc.tile_pool(name="pT", bufs=3) as pTp, \
                 tc.tile_pool(name="partsb", bufs=2) as partsb, \
                 tc.tile_pool(name="ps_s", bufs=2, space="PSUM") as ps_s, \
                 tc.tile_pool(name="ps_o", bufs=2, space="PSUM") as ps_o, \
                 tc.tile_pool(name="ps_rs", bufs=2, space="PSUM") as ps_rs:
                def attn_chunk(j, pair, half):
                    nkt = 2 * (j + 1)
                    pso = ps_o.tile([128, 2 * F], f32, tag="ps_o", name="pso")
                    psr = ps_rs.tile([128, F], f32, tag="ps_rs", name="psr")
                    for ll in range(nkt):
                        pss = ps_s.tile([128, F], f32, tag="ps_s", name="pss")
                        for et in range(2):
                            nc.tensor.matmul(pss[:], kT[et][:, 128 * ll:128 * (ll + 1)],
                                             qT[et][:, 512 * j:512 * (j + 1)],
                                             start=(et == 0), stop=(et == 1))
                        pT = pTp.tile([128, F], bf16, tag="pT", name="pT")
                        nc.scalar.activation(out=pT[:], in_=pss[:], func=Exp)
                        if ll >= nkt - 2:
                            nc.vector.tensor_mul(pT[:], pT[:],
                                                 maskt[:, ll - (nkt - 2), :])
                        for ft in range(2):
                            nc.tensor.matmul(pso[:, F * ft:F * (ft + 1)],
                                             v_sb[:, ll, 128 * ft:128 * (ft + 1)],
                                             pT[:], start=(ll == 0), stop=(ll == nkt - 1),
                                             skip_group_check=True)
                        nc.tensor.matmul(psr[:], ones_r[:], pT[:],
                                         start=(ll == 0), stop=(ll == nkt - 1),
                                         skip_group_check=True)
                    po_sb = partsb.tile([128, 2 * F], f32, tag="po_sb", name="po_sb")
                    nc.scalar.activation(out=po_sb[:], in_=pso[:], func=Copy)
                    pr_sb = partsb.tile([1, F], f32, tag="pr_sb", name="pr_sb")
                    nc.vector.tensor_copy(out=pr_sb[:], in_=psr[0:1, :])
                    for ft in range(2):
                        nc.sync.dma_start(
                            out=partials_in[pair, half, 128 * ft:128 * (ft + 1), :],
                            in_=po_sb[:, F * ft:F * (ft + 1)])
                    nc.sync.dma_start(out=partials_in[pair, half, 256, :], in_=pr_sb[0:1, :])

                for i in range(NPOST):
                    attn_chunk(i, i, 0)
                    attn_chunk(NPOST + i, i, 1)
                    nc.gpsimd.collective_compute(
                        "ReduceScatter", mybir.AluOpType.add,
                        replica_groups=[[0, 1], [2, 3], [4, 5], [6, 7]],
                        ins=[partials_in[i].opt()],
                        outs=[partials_out[i].opt()])

            # ---- post: normalize, transpose back, +bv, store ---------------
            with tc.tile_pool(name="post", bufs=2) as post, \
                 tc.tile_pool(name="ps_po", bufs=4, space="PSUM") as ps_po:
                for c in range(NPOST):
                    oT_sb = post.tile([128, 2 * F], f32, tag="oT_sb")
                    for ft in range(2):
                        nc.sync.dma_start(out=oT_sb[:, F * ft:F * (ft + 1)],
                                          in_=partials_out[c, 128 * ft:128 * (ft + 1), :])
                    rs_ld = post.tile([128, 4], f32, tag="rs_ld")
                    nc.sync.dma_start(out=rs_ld[:],
                                      in_=partials_out[c, 256, :].rearrange("(t p) -> p t", p=128))
                    rs_t = post.tile([128, 4], f32, tag="rs_t")
                    nc.vector.reciprocal(out=rs_t[:], in_=rs_ld[:])
                    onat = post.tile([128, 4, E], f32, tag="onat")
                    for t in range(4):
                        pst = ps_po.tile([128, E], f32, tag="ps_po")
                        for ft in range(2):
                            nc.tensor.transpose(
                                pst[:, 128 * ft:128 * (ft + 1)],
                                oT_sb[:, F * ft + 128 * t:F * ft + 128 * (t + 1)],
                                ident[:])
                        nc.scalar.activation(out=onat[:, t, :], in_=pst[:],
                                             func=Copy, scale=rs_t[:, t:t + 1])
                        nc.vector.tensor_add(onat[:, t, :], onat[:, t, :], bv_bc[:])
                    nc.sync.dma_start(
                        out=out_ext[512 * c:512 * (c + 1), :].rearrange(
                            "(t p) e -> p t e", p=128),
                        in_=onat[:])

    nc.compile()
    return nc


def _get_nc():
    if "nc" not in _COMPILED:
        _COMPILED["nc"] = _build()
    return _COMPILED["nc"]


def kernel(X, Z, mask, Wq, bq, Wk, bk, Wv, bv):
    X = np.asarray(X, dtype=np.float32)
    Z = np.asarray(Z, dtype=np.float32)
    mask_np = np.asarray(mask)

    causal = bool(np.array_equal(
        mask_np != 0, np.tril(np.ones((S, S), dtype=bool))))
    if not causal:
        return _numpy_ref(X, Z, mask_np, Wq, bq, Wk, bk, Wv, bv)

    from concourse.bass_utils import run_bass_kernel_spmd

    nc = _get_nc()

    Wq = np.ascontiguousarray(Wq, dtype=np.float32)
    Wk = np.ascontiguousarray(Wk, dtype=np.float32)
    Wv = np.ascontiguousarray(Wv, dtype=np.float32)
    bqs = (np.asarray(bq, dtype=np.float32) / np.float32(np.sqrt(E))).copy()
    bv = np.ascontiguousarray(bv, dtype=np.float32)
    ones = np.ones((128, 128), dtype=np.float32)

    # masks per parity: last-2 local k-tiles of each chunk; keep iff y >= x+d
    y = np.arange(F)[None, :]
    x = np.arange(128)[:, None]
    masks_par = []
    for p in range(2):
        m = np.stack([(y >= x + 128 * p).astype(np.float32),
                      (y >= x + 256 + 128 * p).astype(np.float32)])
        masks_par.append(np.ascontiguousarray(m))

    in_maps = []
    for c in range(8):
        b, p = c // 2, c % 2
        zb = Z[b].reshape(S // 128, 128, E)
        z_shard = np.ascontiguousarray(zb[p::2].reshape(SK, E))
        in_maps.append({
            "x": np.ascontiguousarray(X[b]),
            "z": z_shard,
            "wq": Wq, "wk": Wk, "wv": Wv,
            "bqs": bqs, "bv": bv,
            "masks": masks_par[p],
            "ones": ones,
        })

    res = run_bass_kernel_spmd(nc, in_maps, core_ids=list(range(8)))

    out = np.empty((B, S, E), dtype=np.float32)
    for b in range(B):
        out[b, :S // 2] = res.results[2 * b]["out"]
        out[b, S // 2:] = res.results[2 * b + 1]["out"]
    return out


def _numpy_ref(X, Z, mask, Wq, bq, Wk, bk, Wv, bv):
    q = np.einsum("bse,fe->bsf", X, Wq) + bq
    k = np.einsum("bse,fe->bsf", Z, Wk) + bk
    v = np.einsum("bse,fe->bsf", Z, Wv) + bv
    s = np.einsum("bqe,bke->bqk", q, k) / np.sqrt(np.float32(X.shape[-1]))
    s = np.where(mask == 0, -np.inf, s)
    s = s - s.max(axis=-1, keepdims=True)
    p = np.exp(s)
    p /= p.sum(axis=-1, keepdims=True)
    return np.einsum("bqk,bke->bqe", p, v).astype(np.float32)


# revision 9
# speedup vs baseline: 1.3164x; 1.0410x over previous
"""Trainium2 Bass kernel: batched causal attention (B=4, S=4096, E=256, f32).

Sharding: 2 cores per batch element (4 pairs). Within a pair, the key/value
rows are split even/odd at 128-row tile granularity; both cores process all
4096 query rows of their batch against their 2048 K/V rows.  This makes the
SPMD instruction stream identical across cores (only data differs) and
perfectly load-balances the causal structure.  Partial (P@V, rowsum) results
are merged across each pair with a ReduceScatter, after which each core
normalizes and writes half the batch rows.

Compute layout (per core):
  X^T, Z^T via PE transposes -> Q^T = WqT @ X^T (scaled by 1/sqrt(E), +bq),
  K^T = WkT @ Z^T (bk dropped: softmax shift-invariant), V = Z^T(stationary)
  @ WvT (bv folded in at the end: attn rows sum to 1).
  Scores computed transposed per tile: S^T[k,q] = K^T(stationary) . Q^T, so
  exp(PSUM)->SBUF directly yields P^T for the PV matmul (no P transpose).
  Rowsums via an M=1 ones-matmul.  Matmuls run in float32r (~1e-4 rel err,
  4x faster than plain f32 on the PE).
"""

import numpy as np

B = 4
S = 4096
E = 256
SK = S // 2          # K/V rows per core
KT = SK // 128       # 16 local k-tiles
NCHUNK = S // 512    # 8 q-chunks of 512
F = 512              # q free dim per chunk
NPOST = NCHUNK // 2  # post-phase chunks per core

_COMPILED = {}


def _build():
    import concourse.bass as bass
    import concourse.tile as tile
    from concourse import mybir, bacc

    f32 = mybir.dt.float32
    f32r = mybir.dt.float32r  # noqa
    bf16 = mybir.dt.bfloat16
    Exp = mybir.ActivationFunctionType.Exp
    Copy = mybir.ActivationFunctionType.Copy
    Ident = mybir.ActivationFunctionType.Identity

    nc = bacc.Bacc("TRN2", target_bir_lowering=False, debug=False,
                   enable_asserts=True, num_devices=8)

    x_ext = nc.dram_tensor("x", [S, E], f32, kind="ExternalInput")
    z_ext = nc.dram_tensor("z", [SK, E], f32, kind="ExternalInput")
    wq_ext = nc.dram_tensor("wq", [E, E], f32, kind="ExternalInput")
    wk_ext = nc.dram_tensor("wk", [E, E], f32, kind="ExternalInput")
    wv_ext = nc.dram_tensor("wv", [E, E], f32, kind="ExternalInput")
    bqs_ext = nc.dram_tensor("bqs", [E], f32, kind="ExternalInput")  # bq/sqrt(E)
    bv_ext = nc.dram_tensor("bv", [E], f32, kind="ExternalInput")
    masks_ext = nc.dram_tensor("masks", [2, 128, F], f32, kind="ExternalInput")
    ones_ext = nc.dram_tensor("ones", [128, 128], f32, kind="ExternalInput")
    out_ext = nc.dram_tensor("out", [S // 2, E], f32, kind="ExternalOutput")

    from concourse.masks import make_identity

    with tile.TileContext(nc) as tc:
        with tc.tile_pool(name="singles", bufs=1) as singles, \
             tc.tile_pool(name="dram", bufs=1, space="DRAM") as dram:
            # ---- constants -------------------------------------------------
            ident = singles.tile([128, 128], f32)
            make_identity(nc, ident[:])
            ident_bf = singles.tile([128, 128], bf16)
            make_identity(nc, ident_bf[:])
            ones_r = singles.tile([128, 128], bf16)
            nc.gpsimd.dma_start(out=ones_r[:], in_=ones_ext[:])
            maskt = singles.tile([128, 2, F], bf16)
            nc.gpsimd.dma_start(out=maskt[:], in_=masks_ext.ap().rearrange("m p f -> p m f"))
            bqs = singles.tile([128, 2], f32)  # [:, ft] per-partition bias
            for ft in range(2):
                nc.sync.dma_start(out=bqs[:, ft:ft + 1],
                                  in_=bqs_ext[128 * ft:128 * (ft + 1)].rearrange("(p one) -> p one", one=1))
            bv_bc = singles.tile([128, E], f32)
            nc.sync.dma_start(
                out=bv_bc[:],
                in_=bass.AP(tensor=bv_ext, offset=0, ap=[[0, 128], [1, E]]))

            # ---- weights: W^T[e', f] in SBUF (f32r), via PE transposes -----
            wT = {}
            with tc.tile_pool(name="wload", bufs=2) as wload, \
                 tc.tile_pool(name="ps_w", bufs=2, space="PSUM") as ps_w:
                for wname, wext in (("q", wq_ext), ("k", wk_ext), ("v", wv_ext)):
                    for et in range(2):
                        wT[wname, et] = singles.tile([128, E], bf16, name=f"wT_{wname}{et}")
                    for ft in range(2):
                        wnat = wload.tile([128, E], bf16)
                        nc.gpsimd.dma_start(out=wnat[:],
                                            in_=wext[128 * ft:128 * (ft + 1), :])
                        pst = ps_w.tile([128, E], bf16)
                        for et in range(2):
                            nc.tensor.transpose(pst[:, 128 * et:128 * (et + 1)],
                                                wnat[:, 128 * et:128 * (et + 1)],
                                                ident_bf[:])
                        for et in range(2):
                            nc.vector.tensor_copy(
                                out=wT[wname, et][:, 128 * ft:128 * (ft + 1)],
                                in_=pst[:, 128 * et:128 * (et + 1)])

            # ---- big persistent SBUF tensors -------------------------------
            qT = [singles.tile([128, S], bf16, name=f"qT{i}", tag=f"qT{i}") for i in range(2)]
            kT = [singles.tile([128, SK], bf16, name=f"kT{i}", tag=f"kT{i}") for i in range(2)]
            v_sb = singles.tile([128, KT, E], bf16, tag="v_sb")

            # ---- projections ----------------------------------------------
            with tc.tile_pool(name="nat", bufs=3) as nat, \
                 tc.tile_pool(name="trsb", bufs=4) as trsb, \
                 tc.tile_pool(name="ps_tr", bufs=3, space="PSUM") as ps_tr, \
                 tc.tile_pool(name="ps_mm", bufs=3, space="PSUM") as ps_mm:
                # Z^T chunks -> K^T, V
                for sc in range(4):
                    z_nat = nat.tile([128, 4, E], bf16, tag="nat")
                    nc.gpsimd.dma_start(
                        out=z_nat[:],
                        in_=z_ext[512 * sc:512 * (sc + 1), :].rearrange(
                            "(t p) e -> p t e", p=128))
                    zT = []
                    for et in range(2):
                        pst = ps_tr.tile([128, F], bf16, tag="ps_tr")
                        for t in range(4):
                            nc.tensor.transpose(
                                pst[:, 128 * t:128 * (t + 1)],
                                z_nat[:, t, 128 * et:128 * (et + 1)], ident_bf[:])
                        zt = trsb.tile([128, F], bf16, tag="xT")
                        nc.vector.tensor_copy(out=zt[:], in_=pst[:])
                        zT.append(zt)
                    for ft in range(2):
                        psk = ps_mm.tile([128, F], f32, tag="ps_mm")
                        for et in range(2):
                            nc.tensor.matmul(psk[:], wT["k", et][:, 128 * ft:128 * (ft + 1)],
                                             zT[et][:], start=(et == 0), stop=(et == 1))
                        nc.vector.tensor_copy(out=kT[ft][:, 512 * sc:512 * (sc + 1)],
                                              in_=psk[:])
                    for t in range(4):
                        psv = ps_mm.tile([128, E], f32, tag="ps_mm")
                        for et in range(2):
                            nc.tensor.matmul(psv[:], zT[et][:, 128 * t:128 * (t + 1)],
                                             wT["v", et][:], start=(et == 0), stop=(et == 1))
                        nc.vector.tensor_copy(out=v_sb[:, 4 * sc + t, :], in_=psv[:])

                # X^T chunks -> Q^T
                for sc in (0, 4, 1, 5, 2, 6, 3, 7):
                    x_nat = nat.tile([128, 4, E], bf16, tag="nat")
                    nc.gpsimd.dma_start(
                        out=x_nat[:],
                        in_=x_ext[512 * sc:512 * (sc + 1), :].rearrange(
                            "(t p) e -> p t e", p=128))
                    xT = []
                    for et in range(2):
                        pst = ps_tr.tile([128, F], bf16, tag="ps_tr")
                        for t in range(4):
                            nc.tensor.transpose(
                                pst[:, 128 * t:128 * (t + 1)],
                                x_nat[:, t, 128 * et:128 * (et + 1)], ident_bf[:])
                        xt = trsb.tile([128, F], bf16, tag="xT")
                        nc.vector.tensor_copy(out=xt[:], in_=pst[:])
                        xT.append(xt)
                    for ft in range(2):
                        psq = ps_mm.tile([128, F], f32, tag="ps_mm")
                        for et in range(2):
                            nc.tensor.matmul(psq[:], wT["q", et][:, 128 * ft:128 * (ft + 1)],
                                             xT[et][:], start=(et == 0), stop=(et == 1))
                        nc.scalar.activation(out=qT[ft][:, 512 * sc:512 * (sc + 1)],
                                             in_=psq[:], func=Ident,
                                             bias=bqs[:, ft:ft + 1],
                                             scale=1.0 / np.sqrt(E))
            # ---- attention -------------------------------------------------
            partials_in = dram.tile([NPOST, 2, 257, F], f32)
            partials_out = dram.tile([NPOST, 257, F], f32)

            with tc.tile_pool(name="pT", bufs=3) as pTp, \
                 tc.tile_pool(name="partsb", bufs=2) as partsb, \
                 tc.tile_pool(name="ps_s", bufs=4, space="PSUM") as ps_s, \
                 tc.tile_pool(name="ps_o", bufs=1, space="PSUM") as ps_o, \
                 tc.tile_pool(name="ps_rs", bufs=2, space="PSUM") as ps_rs:
                def attn_chunk(j, pair, half):
                    nkt = 2 * (j + 1)
                    pso = ps_o.tile([128, 2 * F], f32, tag="ps_o", name="pso")
                    psr = ps_rs.tile([128, F], f32, tag="ps_rs", name="psr")
                    for ll in range(nkt):
                        pss = ps_s.tile([128, F], f32, tag="ps_s", name="pss")
                        for et in range(2):
                            nc.tensor.matmul(pss[:], kT[et][:, 128 * ll:128 * (ll + 1)],
                                             qT[et][:, 512 * j:512 * (j + 1)],
                                             start=(et == 0), stop=(et == 1))
                        pT = pTp.tile([128, F], bf16, tag="pT", name="pT")
                        nc.scalar.activation(out=pT[:], in_=pss[:], func=Exp)
                        if ll >= nkt - 2:
                            nc.vector.tensor_mul(pT[:], pT[:],
                                                 maskt[:, ll - (nkt - 2), :])
                        for ft in range(2):
                            nc.tensor.matmul(pso[:, F * ft:F * (ft + 1)],
                                             v_sb[:, ll, 128 * ft:128 * (ft + 1)],
                                             pT[:], start=(ll == 0), stop=(ll == nkt - 1),
                                             skip_group_check=True)
                        nc.tensor.matmul(psr[:], ones_r[:], pT[:],
                                         start=(ll == 0), stop=(ll == nkt - 1),
                                         skip_group_check=True)
                    po_sb = partsb.tile([128, 2 * F], f32, tag="po_sb", name="po_sb")
                    nc.scalar.activation(out=po_sb[:], in_=pso[:], func=Copy)
                    pr_sb = partsb.tile([1, F], f32, tag="pr_sb", name="pr_sb")
                    nc.vector.tensor_copy(out=pr_sb[:], in_=psr[0:1, :])
                    for ft in range(2):
                        nc.sync.dma_start(
                            out=partials_in[pair, half, 128 * ft:128 * (ft + 1), :],
                            in_=po_sb[:, F * ft:F * (ft + 1)])
                    nc.sync.dma_start(out=partials_in[pair, half, 256, :], in_=pr_sb[0:1, :])

                for i in range(NPOST):
                    attn_chunk(i, i, 0)
                    attn_chunk(NPOST + i, i, 1)
                    nc.gpsimd.collective_compute(
                        "ReduceScatter", mybir.AluOpType.add,
                        replica_groups=[[0, 1], [2, 3], [4, 5], [6, 7]],
                        ins=[partials_in[i].opt()],
                        outs=[partials_out[i].opt()])

            # ---- post: normalize, transpose back, +bv, store ---------------
            with tc.tile_pool(name="post", bufs=2) as post, \
                 tc.tile_pool(name="ps_po", bufs=4, space="PSUM") as ps_po:
                for c in range(NPOST):
                    oT_sb = post.tile([128, 2 * F], f32, tag="oT_sb")
                    for ft in range(2):
                        nc.sync.dma_start(out=oT_sb[:, F * ft:F * (ft + 1)],
                                          in_=partials_out[c, 128 * ft:128 * (ft + 1), :])
                    rs_ld = post.tile([128, 4], f32, tag="rs_ld")
                    nc.sync.dma_start(out=rs_ld[:],
                                      in_=partials_out[c, 256, :].rearrange("(t p) -> p t", p=128))
                    rs_t = post.tile([128, 4], f32, tag="rs_t")
                    nc.vector.reciprocal(out=rs_t[:], in_=rs_ld[:])
                    onat = post.tile([128, 4, E], f32, tag="onat")
                    for t in range(4):
                        pst = ps_po.tile([128, E], f32, tag="ps_po")
                        for ft in range(2):
                            nc.tensor.transpose(
                                pst[:, 128 * ft:128 * (ft + 1)],
                                oT_sb[:, F * ft + 128 * t:F * ft + 128 * (t + 1)],
                                ident[:])
                        nc.scalar.activation(out=onat[:, t, :], in_=pst[:],
                                             func=Copy, scale=rs_t[:, t:t + 1])
                        nc.vector.tensor_add(onat[:, t, :], onat[:, t, :], bv_bc[:])
                    nc.sync.dma_start(
                        out=out_ext[512 * c:512 * (c + 1), :].rearrange(
                            "(t p) e -> p t e", p=128),
                        in_=onat[:])

    nc.compile()
    return nc


def _get_nc():
    if "nc" not in _COMPILED:
        _COMPILED["nc"] = _build()
    return _COMPILED["nc"]


def kernel(X, Z, mask, Wq, bq, Wk, bk, Wv, bv):
    X = np.asarray(X, dtype=np.float32)
    Z = np.asarray(Z, dtype=np.float32)
    mask_np = np.asarray(mask)

    causal = bool(np.array_equal(
        mask_np != 0, np.tril(np.ones((S, S), dtype=bool))))
    if not causal:
        return _numpy_ref(X, Z, mask_np, Wq, bq, Wk, bk, Wv, bv)

    from concourse.bass_utils import run_bass_kernel_spmd

    nc = _get_nc()

    Wq = np.ascontiguousarray(Wq, dtype=np.float32)
    Wk = np.ascontiguousarray(Wk, dtype=np.float32)
    Wv = np.ascontiguousarray(Wv, dtype=np.float32)
    bqs = (np.asarray(bq, dtype=np.float32) / np.float32(np.sqrt(E))).copy()
    bv = np.ascontiguousarray(bv, dtype=np.float32)
    ones = np.ones((128, 128), dtype=np.float32)

    # masks per parity: last-2 local k-tiles of each chunk; keep iff y >= x+d
    y = np.arange(F)[None, :]
    x = np.arange(128)[:, None]
    masks_par = []
    for p in range(2):
        m = np.stack([(y >= x + 128 * p).astype(np.float32),
                      (y >= x + 256 + 128 * p).astype(np.float32)])
        masks_par.append(np.ascontiguousarray(m))

    in_maps = []
    for c in range(8):
        b, p = c // 2, c % 2
        zb = Z[b].reshape(S // 128, 128, E)
        z_shard = np.ascontiguousarray(zb[p::2].reshape(SK, E))
        in_maps.append({
            "x": np.ascontiguousarray(X[b]),
            "z": z_shard,
            "wq": Wq, "wk": Wk, "wv": Wv,
            "bqs": bqs, "bv": bv,
            "masks": masks_par[p],
            "ones": ones,
        })

    res = run_bass_kernel_spmd(nc, in_maps, core_ids=list(range(8)))

    out = np.empty((B, S, E), dtype=np.float32)
    for b in range(B):
        out[b, :S // 2] = res.results[2 * b]["out"]
        out[b, S // 2:] = res.results[2 * b + 1]["out"]
    return out


def _numpy_ref(X, Z, mask, Wq, bq, Wk, bk, Wv, bv):
    q = np.einsum("bse,fe->bsf", X, Wq) + bq
    k = np.einsum("bse,fe->bsf", Z, Wk) + bk
    v = np.einsum("bse,fe->bsf", Z, Wv) + bv
    s = np.einsum("bqe,bke->bqk", q, k) / np.sqrt(np.float32(X.shape[-1]))
    s = np.where(mask == 0, -np.inf, s)
    s = s - s.max(axis=-1, keepdims=True)
    p = np.exp(s)
    p /= p.sum(axis=-1, keepdims=True)
    return np.einsum("bqk,bke->bqe", p, v).astype(np.float32)
